# revision 5
# baseline (speedup 1.0000x reference)
"""Causal sliding-window attention (T=8192, H=16, HK=4, D=128, W=512) on 8 trn2 cores.

Sharding: tensor-parallel on heads. Core c computes query heads {2c, 2c+1},
which share kv head c//2 (G = H//HK = 4, and 2 heads per core never straddle
a kv group). Each core is fully independent -- no collectives.

Per-core program (Bass/Tile, SPMD):
  inputs (host pre-transposed, pre-cast bf16):
    qT  [2*128, T]   Q^T per head (row block j = head j)
    kT  [128, T]     K^T of the shared kv head
    va  [128, nT*129] V chunks [128, 129] with a ones column (chunk t at cols
                     129t..129t+129); the ones column makes the PV matmul also
                     produce the softmax denominator.
  loop over k-chunks t (128 keys each), keys on PSUM partitions:
    S^T[rk, q] = kT_chunk.T @ qT  over the 640-wide valid q-span [128t, 128t+640)
    one ACT exp (scale=D^-0.5 folded in), fp32->bf16, into an SBUF ring
    triangular edge masks (diag block j=0, border block j=4) via one DVE mul
    PV: for j=0..4, lhsT = E block (q-chunk t+j), rhs = va chunk t -> accumulate
        O_aug[q-chunk] = [128, 129] in PSUM (col 128 = denominator)
    retire q-chunk t: recip(denom) -> staged, O * recip -> out staging -> HBM
  lse computed on host as -log(recip).

PSUM (8 banks): 4 banks = S ring-of-3 [128, 1920]; 4 banks = O accumulators,
5 live slots with staggered lifetimes packed 2 per bank at offsets 0/129 using
(u, u+4) co-tenancy: bank u%4, offset (u//4)%2. start=True (whole-bank
has_written clear) is only ever issued by the offset-0 occupant at its first
touch, which is exactly when the other offset's previous occupant has retired.
"""

import numpy as np
import ml_dtypes

import concourse.bacc as bacc
import concourse.bass as bass
import concourse.mybir as mybir
import concourse.tile as tile
from concourse.bass_utils import run_bass_kernel_spmd

T, H, HK, D, W = 8192, 16, 4, 128, 512
NCORES = 8
SCALE = float(D) ** -0.5
BF16 = ml_dtypes.bfloat16
F32 = mybir.dt.float32
BF = mybir.dt.bfloat16

_NC_CACHE = {}


def _split_at_banks(col0, width):
    """Split [col0, col0+width) PSUM cols at 512 boundaries."""
    pieces = []
    c = col0
    end = col0 + width
    while c < end:
        nxt = min(end, (c // 512 + 1) * 512)
        pieces.append((c, nxt - c))
        c = nxt
    return pieces


def build_program(t_tokens=T):
    nT = t_tokens // 128  # number of 128-wide k/q chunks
    assert t_tokens % 512 == 0 and nT >= 8

    nc = bacc.Bacc("TRN2", target_bir_lowering=False, debug=False,
                   num_devices=NCORES)
    qT_d = nc.dram_tensor("qT", [2 * D, t_tokens], BF, kind="ExternalInput")
    kT_d = nc.dram_tensor("kT", [D, t_tokens], BF, kind="ExternalInput")
    va_d = nc.dram_tensor("va", [D, nT * 129], BF, kind="ExternalInput")
    out_d = nc.dram_tensor("out", [t_tokens, 2 * D], F32, kind="ExternalOutput")
    rcp_d = nc.dram_tensor("rcp", [D, 2 * nT], F32, kind="ExternalOutput")

    with tile.TileContext(nc) as tc:
        with (
            tc.tile_pool(name="resident", bufs=1) as rpool,
            tc.tile_pool(name="ostg", bufs=2) as ostg_pool,
            tc.tile_pool(name="psum", bufs=1, space="PSUM") as psum_pool,
        ):
            kT = rpool.tile([128, t_tokens], BF, tag="kT")
            va = rpool.tile([128, nT * 129], BF, tag="va")
            qT = [rpool.tile([128, t_tokens], BF, tag=f"qT{j}", name=f"qT{j}")
                  for j in range(2)]
            masks = rpool.tile([128, 256], BF, tag="masks")
            den = rpool.tile([128, 2 * nT], F32, tag="den")
            ering = rpool.tile([128, 1920], BF, tag="ering")

            sring = psum_pool.tile([128, 1920], F32, tag="sring")
            oacc = psum_pool.tile([128, 2048], F32, tag="oacc")

            # --- constant masks (bf16 0/1) ---
            # cols [0,128): diagonal block, valid iff rk <= cq
            # cols [128,256): border block, valid iff rk > cq
            nc.gpsimd.memset(masks[:, :], 1.0)
            nc.gpsimd.affine_select(
                out=masks[:, 0:128], in_=masks[:, 0:128],
                compare_op=mybir.AluOpType.is_ge, fill=0.0,
                base=0, channel_multiplier=-1, pattern=[[1, 128]],
            )
            nc.gpsimd.affine_select(
                out=masks[:, 128:256], in_=masks[:, 128:256],
                compare_op=mybir.AluOpType.is_ge, fill=0.0,
                base=-1, channel_multiplier=1, pattern=[[-1, 128]],
            )

            # --- input DMA, sliced for overlap with compute ---
            ks = t_tokens // 4
            for s in range(4):
                nc.sync.dma_start(kT[:, s * ks:(s + 1) * ks],
                                  kT_d[:, s * ks:(s + 1) * ks])
            vs = (nT * 129) // 4
            for s in range(4):
                nc.sync.dma_start(va[:, s * vs:(s + 1) * vs],
                                  va_d[:, s * vs:(s + 1) * vs])
            qs = t_tokens // 8
            for j in range(2):
                for s in range(8):
                    nc.sync.dma_start(qT[j][:, s * qs:(s + 1) * qs],
                                      qT_d[128 * j:128 * j + 128,
                                           s * qs:(s + 1) * qs])

            out_r = out_d.rearrange("(b c p) e -> b p c e", c=4, p=128)

            def s_matmuls(j, t):
                """S^T matmuls for k-chunk t into ring slice t%3."""
                base = 640 * (t % 3)
                k0 = 128 * t
                wm = min(512, t_tokens - k0)
                for (c0, w) in _split_at_banks(base, wm):
                    qc = k0 + (c0 - base)
                    nc.tensor.matmul(sring[:, c0:c0 + w],
                                     kT[:, k0:k0 + 128],
                                     qT[j][:, qc:qc + w],
                                     start=True, stop=True)
                wt = min(128, max(0, t_tokens - k0 - 512))
                if wt > 0:
                    c0 = base + 512
                    nc.tensor.matmul(sring[:, c0:c0 + wt],
                                     kT[:, k0:k0 + 128],
                                     qT[j][:, k0 + 512:k0 + 512 + wt],
                                     start=True, stop=True)
                return wm + wt

            def emit_exp(t, width):
                base = 640 * (t % 3)
                nc.scalar.activation(ering[:, base:base + width],
                                     sring[:, base:base + width],
                                     mybir.ActivationFunctionType.Exp,
                                     scale=SCALE)

            def emit_exp_pair(t, w0, w1):
                """exp over ring slices t%3 and (t+1)%3; one call if adjacent."""
                a = t % 3
                if a <= 1 and w0 == 640:
                    nc.scalar.activation(ering[:, 640 * a:640 * a + 640 + w1],
                                         sring[:, 640 * a:640 * a + 640 + w1],
                                         mybir.ActivationFunctionType.Exp,
                                         scale=SCALE)
                else:
                    emit_exp(t, w0)
                    emit_exp(t + 1, w1)

            def mask_and_pv(j, t):
                base = 640 * (t % 3)
                has_border = (128 * t + 640) <= t_tokens
                # edge masks: diag block always; border block only if present
                if has_border:
                    e2 = ering[:, base:base + 640].rearrange(
                        "p (b c) -> p b c", c=128)[:, 0:5:4, :]
                    m2 = masks.rearrange("p (b c) -> p b c", c=128)
                    nc.vector.tensor_mul(e2, e2, m2)
                else:
                    nc.vector.tensor_mul(ering[:, base:base + 128],
                                         ering[:, base:base + 128],
                                         masks[:, 0:128])
                # PV, ascending j so same-bank clears follow last accumulations
                jmax = min(4, nT - 1 - t)
                for jj in range(jmax + 1):
                    u = t + jj
                    off = 512 * (u % 4) + 129 * ((u // 4) % 2)
                    first = (jj == 4) or (t == 0)
                    start = first and ((u // 4) % 2 == 0)
                    # Co-tenant accumulators share banks; the sim's
                    # bank-granular group checker can't express this, but its
                    # per-byte pending-zero value model (== HW has_written)
                    # verifies the numerics.
                    nc.tensor.matmul(
                        oacc[:, off:off + 129],
                        ering[:, base + 128 * jj:base + 128 * jj + 128],
                        va[:, 129 * t:129 * t + 129],
                        start=start, stop=(jj == 0),
                        skip_group_check=True)

            ostage = [None]

            def retire(j, t):
                u = t
                off = 512 * (u % 4) + 129 * ((u // 4) % 2)
                dcol = den[:, nT * j + t:nT * j + t + 1]
                nc.vector.reciprocal(dcol, oacc[:, off + 128:off + 129])
                if t % 4 == 0:
                    ostage[0] = ostg_pool.tile([128, 512], F32, tag="ostage",
                                               name="ostage")
                nc.vector.tensor_scalar_mul(
                    ostage[0][:, 128 * (t % 4):128 * (t % 4) + 128],
                    oacc[:, off:off + 128], dcol)
                if t % 4 == 3:
                    blk = t // 4
                    src = ostage[0].rearrange("p (c e) -> p c e", e=128)
                    dst = out_r[blk, :, :, 128 * j:128 * j + 128]
                    nc.sync.dma_start(dst, src)

            for j in range(2):
                t = 0
                while t < nT:
                    pairable = (t + 1 < nT) and (128 * t + 640 <= t_tokens)
                    if pairable:
                        w0 = s_matmuls(j, t)
                        w1 = s_matmuls(j, t + 1)
                        emit_exp_pair(t, w0, w1)
                        mask_and_pv(j, t)
                        retire(j, t)
                        mask_and_pv(j, t + 1)
                        retire(j, t + 1)
                        t += 2
                    else:
                        w0 = s_matmuls(j, t)
                        emit_exp(t, w0)
                        mask_and_pv(j, t)
                        retire(j, t)
                        t += 1

            nc.sync.dma_start(rcp_d[:, :], den[:, :])

    nc.compile()
    return nc


def _get_nc(t_tokens=T):
    if t_tokens not in _NC_CACHE:
        _NC_CACHE[t_tokens] = build_program(t_tokens)
    return _NC_CACHE[t_tokens]


def make_in_maps(query, key, value, t_tokens=T):
    q = np.asarray(query).astype(BF16).reshape(t_tokens, H, D)
    k = np.asarray(key).astype(BF16).reshape(t_tokens, HK, D)
    v = np.asarray(value).astype(BF16).reshape(t_tokens, HK, D)
    nT = t_tokens // 128
    in_maps = []
    for c in range(NCORES):
        h0, hk = 2 * c, c // 2
        qT = np.ascontiguousarray(
            q[:, h0:h0 + 2, :].transpose(1, 2, 0)).reshape(2 * D, t_tokens)
        kT = np.ascontiguousarray(k[:, hk, :].T)
        vv = v[:, hk, :].reshape(nT, 128, D).transpose(1, 0, 2)
        va = np.empty((128, nT, D + 1), dtype=BF16)
        va[:, :, :D] = vv
        va[:, :, D] = 1.0
        in_maps.append({"qT": qT, "kT": kT, "va": va.reshape(128, nT * 129)})
    return in_maps


def assemble(results, t_tokens=T):
    nT = t_tokens // 128
    out = np.empty((t_tokens, H * D), np.float32)
    lse = np.empty((H, t_tokens), np.float32)
    for c in range(NCORES):
        r = results[c]
        out[:, 256 * c:256 * (c + 1)] = r["out"]
        rcp = r["rcp"]
        for j in range(2):
            lse[2 * c + j] = -np.log(
                rcp[:, nT * j:nT * (j + 1)].T.reshape(-1))
    return out, lse


def kernel(query, key, value):
    nc = _get_nc(T)
    in_maps = make_in_maps(query, key, value, T)
    res = run_bass_kernel_spmd(nc, in_maps, list(range(NCORES)))
    return assemble(res.results, T)


# revision 6
# speedup vs baseline: 1.0251x; 1.0251x over previous
"""Causal sliding-window attention (T=8192, H=16, HK=4, D=128, W=512) on 8 trn2 cores.

Sharding: tensor-parallel on heads. Core c computes query heads {2c, 2c+1},
which share kv head c//2 (G = H//HK = 4, so 2 heads per core never straddle
a kv group). Each core is fully independent -- no collectives.

Per-core program (Bass/Tile, SPMD):
  inputs (host pre-transposed, pre-cast bf16):
    qT  [2*128, T]   Q^T per head (row block j = head j)
    kT  [128, T]     K^T of the shared kv head
    va  [128, nT*129] V chunks [128, 129] with a ones column (chunk t at cols
                     129t..129t+129); the ones column makes the PV matmul also
                     produce the softmax denominator.
  loop over k-chunks t (128 keys each), keys on PSUM partitions:
    S^T[rk, q] = kT_chunk.T @ qT  over the 640-wide valid q-span [128t, 128t+640)
    one ACT exp (scale=D^-0.5 folded in), fp32->bf16, into an SBUF ring
    triangular edge masks (diag block on DVE, border block on GpSimd)
    PV: for j=0..4, lhsT = E block (q-chunk t+j), rhs = va chunk t -> accumulate
        O_aug[q-chunk] = [128, 129] in PSUM (col 128 = denominator)
    retire q-chunk t: copy unnormalized O and denominator to SBUF staging.
  Normalization (out/den) and lse (log den) happen on the host.

The emission is software-pipelined (S matmuls run 2 k-steps ahead of exp) so
the in-order PE queue never stalls on ACT: this keeps PE busy continuously,
which also keeps the PE HAM clock-gate at 2.4 GHz.

PSUM (8 banks): 4 banks = S ring-of-3 [128, 1920]; 4 banks = O accumulators,
5 live slots with staggered lifetimes packed 2 per bank at offsets 0/129 using
(u, u+4) co-tenancy: bank u%4, offset (u//4)%2. start=True (whole-bank
has_written clear) is only ever issued by the offset-0 occupant at its first
touch, which is exactly when the other offset's previous occupant has retired.
"""

import numpy as np
import ml_dtypes

import concourse.bacc as bacc
import concourse.bass as bass
import concourse.mybir as mybir
import concourse.tile as tile
from concourse.bass_utils import run_bass_kernel_spmd

T, H, HK, D, W = 8192, 16, 4, 128, 512
NCORES = 8
SCALE = float(D) ** -0.5
BF16 = ml_dtypes.bfloat16
F32 = mybir.dt.float32
BF = mybir.dt.bfloat16

_NC_CACHE = {}


def _split_at_banks(col0, width):
    """Split [col0, col0+width) PSUM cols at 512 boundaries."""
    pieces = []
    c = col0
    end = col0 + width
    while c < end:
        nxt = min(end, (c // 512 + 1) * 512)
        pieces.append((c, nxt - c))
        c = nxt
    return pieces


def build_program(t_tokens=T):
    nT = t_tokens // 128  # number of 128-wide k/q chunks
    assert t_tokens % 512 == 0 and nT >= 8

    nc = bacc.Bacc("TRN2", target_bir_lowering=False, debug=False,
                   num_devices=NCORES)
    qT_d = nc.dram_tensor("qT", [2 * D, t_tokens], BF, kind="ExternalInput")
    kT_d = nc.dram_tensor("kT", [D, t_tokens], BF, kind="ExternalInput")
    va_d = nc.dram_tensor("va", [D, nT * 129], BF, kind="ExternalInput")
    out_d = nc.dram_tensor("out", [t_tokens, 2 * D], F32, kind="ExternalOutput")
    den_d = nc.dram_tensor("den", [D, 2 * nT], F32, kind="ExternalOutput")

    with tile.TileContext(nc) as tc:
        with (
            tc.tile_pool(name="resident", bufs=1) as rpool,
            tc.tile_pool(name="ostg", bufs=2) as ostg_pool,
            tc.tile_pool(name="psum", bufs=1, space="PSUM") as psum_pool,
        ):
            kT = rpool.tile([128, t_tokens], BF, tag="kT")
            va = rpool.tile([128, nT * 129], BF, tag="va")
            qT = [rpool.tile([128, t_tokens], BF, tag=f"qT{j}", name=f"qT{j}")
                  for j in range(2)]
            masks = rpool.tile([128, 256], BF, tag="masks")
            den = rpool.tile([128, 2 * nT], F32, tag="den")
            ering = rpool.tile([128, 1920], BF, tag="ering")

            sring = psum_pool.tile([128, 1920], F32, tag="sring")
            oacc = psum_pool.tile([128, 2048], F32, tag="oacc")

            # --- constant masks (bf16 0/1) ---
            # cols [0,128): diagonal block, valid iff rk <= cq
            # cols [128,256): border block, valid iff rk > cq
            nc.gpsimd.memset(masks[:, :], 1.0)
            nc.gpsimd.affine_select(
                out=masks[:, 0:128], in_=masks[:, 0:128],
                compare_op=mybir.AluOpType.is_ge, fill=0.0,
                base=0, channel_multiplier=-1, pattern=[[1, 128]],
            )
            nc.gpsimd.affine_select(
                out=masks[:, 128:256], in_=masks[:, 128:256],
                compare_op=mybir.AluOpType.is_ge, fill=0.0,
                base=-1, channel_multiplier=1, pattern=[[-1, 128]],
            )

            # --- input DMA, sliced and interleaved in first-use order ---
            ks = t_tokens // 4
            vs = (nT * 129) // 4
            qs = t_tokens // 8
            for s in range(4):
                nc.sync.dma_start(kT[:, s * ks:(s + 1) * ks],
                                  kT_d[:, s * ks:(s + 1) * ks])
                nc.sync.dma_start(qT[0][:, s * qs:(s + 1) * qs],
                                  qT_d[0:128, s * qs:(s + 1) * qs])
                nc.sync.dma_start(va[:, s * vs:(s + 1) * vs],
                                  va_d[:, s * vs:(s + 1) * vs])
            for s in range(4, 8):
                nc.sync.dma_start(qT[0][:, s * qs:(s + 1) * qs],
                                  qT_d[0:128, s * qs:(s + 1) * qs])
            for s in range(8):
                nc.sync.dma_start(qT[1][:, s * qs:(s + 1) * qs],
                                  qT_d[128:256, s * qs:(s + 1) * qs])

            out_r = out_d.rearrange("(b c p) e -> b p c e", c=4, p=128)

            def s_matmuls(j, t):
                """S^T matmuls for k-chunk t into ring slice t%3."""
                base = 640 * (t % 3)
                k0 = 128 * t
                wm = min(512, t_tokens - k0)
                for (c0, w) in _split_at_banks(base, wm):
                    qc = k0 + (c0 - base)
                    nc.tensor.matmul(sring[:, c0:c0 + w],
                                     kT[:, k0:k0 + 128],
                                     qT[j][:, qc:qc + w],
                                     start=True, stop=True)
                wt = min(128, max(0, t_tokens - k0 - 512))
                if wt > 0:
                    c0 = base + 512
                    nc.tensor.matmul(sring[:, c0:c0 + wt],
                                     kT[:, k0:k0 + 128],
                                     qT[j][:, k0 + 512:k0 + 512 + wt],
                                     start=True, stop=True)
                return wm + wt

            def width_of(t):
                k0 = 128 * t
                return min(512, t_tokens - k0) + min(128, max(0, t_tokens - k0 - 512))

            def emit_exp(t):
                base = 640 * (t % 3)
                w = width_of(t)
                nc.scalar.activation(ering[:, base:base + w],
                                     sring[:, base:base + w],
                                     mybir.ActivationFunctionType.Exp,
                                     scale=SCALE)

            def mask_and_pv(j, t):
                base = 640 * (t % 3)
                has_border = (128 * t + 640) <= t_tokens
                # diag mask on DVE (4x bf16 mode), border mask on GpSimd
                nc.vector.tensor_mul(ering[:, base:base + 128],
                                     ering[:, base:base + 128],
                                     masks[:, 0:128])
                if has_border:
                    nc.gpsimd.tensor_mul(ering[:, base + 512:base + 640],
                                         ering[:, base + 512:base + 640],
                                         masks[:, 128:256])
                # PV, ascending j so same-bank clears follow last accumulations
                jmax = min(4, nT - 1 - t)
                for jj in range(jmax + 1):
                    u = t + jj
                    off = 512 * (u % 4) + 129 * ((u // 4) % 2)
                    first = (jj == 4) or (t == 0)
                    start = first and ((u // 4) % 2 == 0)
                    # Co-tenant accumulators share banks; the sim's
                    # bank-granular group checker can't express this, but its
                    # per-byte pending-zero value model (== HW has_written)
                    # verifies the numerics.
                    nc.tensor.matmul(
                        oacc[:, off:off + 129],
                        ering[:, base + 128 * jj:base + 128 * jj + 128],
                        va[:, 129 * t:129 * t + 129],
                        start=start, stop=(jj == 0),
                        skip_group_check=True)

            ostage = [None]

            def retire(j, t):
                u = t
                off = 512 * (u % 4) + 129 * ((u // 4) % 2)
                nc.vector.tensor_copy(den[:, nT * j + t:nT * j + t + 1],
                                      oacc[:, off + 128:off + 129])
                if t % 4 == 0:
                    ostage[0] = ostg_pool.tile([128, 512], F32, tag="ostage",
                                               name="ostage")
                nc.vector.tensor_copy(
                    ostage[0][:, 128 * (t % 4):128 * (t % 4) + 128],
                    oacc[:, off:off + 128])
                if t % 4 == 3:
                    blk = t // 4
                    src = ostage[0].rearrange("p (c e) -> p c e", e=128)
                    dst = out_r[blk, :, :, 128 * j:128 * j + 128]
                    nc.sync.dma_start(dst, src)

            # software-pipelined emission: S runs 2 k-steps ahead of exp
            for j in range(2):
                s_matmuls(j, 0)
                s_matmuls(j, 1)
                for t in range(nT):
                    emit_exp(t)
                    if t + 2 < nT:
                        s_matmuls(j, t + 2)
                    mask_and_pv(j, t)
                    retire(j, t)

            nc.sync.dma_start(den_d[:, :], den[:, :])

    nc.compile()
    return nc


def _get_nc(t_tokens=T):
    if t_tokens not in _NC_CACHE:
        _NC_CACHE[t_tokens] = build_program(t_tokens)
    return _NC_CACHE[t_tokens]


def make_in_maps(query, key, value, t_tokens=T):
    q = np.asarray(query).astype(BF16).reshape(t_tokens, H, D)
    k = np.asarray(key).astype(BF16).reshape(t_tokens, HK, D)
    v = np.asarray(value).astype(BF16).reshape(t_tokens, HK, D)
    nT = t_tokens // 128
    in_maps = []
    for c in range(NCORES):
        h0, hk = 2 * c, c // 2
        qT = np.ascontiguousarray(
            q[:, h0:h0 + 2, :].transpose(1, 2, 0)).reshape(2 * D, t_tokens)
        kT = np.ascontiguousarray(k[:, hk, :].T)
        vv = v[:, hk, :].reshape(nT, 128, D).transpose(1, 0, 2)
        va = np.empty((128, nT, D + 1), dtype=BF16)
        va[:, :, :D] = vv
        va[:, :, D] = 1.0
        in_maps.append({"qT": qT, "kT": kT, "va": va.reshape(128, nT * 129)})
    return in_maps


def assemble(results, t_tokens=T):
    nT = t_tokens // 128
    out = np.empty((t_tokens, H * D), np.float32)
    lse = np.empty((H, t_tokens), np.float32)
    for c in range(NCORES):
        r = results[c]
        o = r["out"]
        dn = r["den"]
        for j in range(2):
            d_q = dn[:, nT * j:nT * (j + 1)].T.reshape(-1)  # [T] per-query
            cols = slice(256 * c + 128 * j, 256 * c + 128 * j + 128)
            out[:, cols] = o[:, 128 * j:128 * j + 128] / d_q[:, None]
            lse[2 * c + j] = np.log(d_q)
    return out, lse


def kernel(query, key, value):
    nc = _get_nc(T)
    in_maps = make_in_maps(query, key, value, T)
    res = run_bass_kernel_spmd(nc, in_maps, list(range(NCORES)))
    return assemble(res.results, T)


# revision 7
# speedup vs baseline: 1.2330x; 1.2028x over previous
"""Causal sliding-window attention (T=8192, H=16, HK=4, D=128, W=512) on 8 trn2 cores.

Sharding: tensor-parallel on heads. Core c computes query heads {2c, 2c+1},
which share kv head c//2 (G = H//HK = 4, so 2 heads per core never straddle
a kv group). Each core is fully independent -- no collectives.

Per-core program (Bass/Tile, SPMD):
  inputs (host pre-transposed, pre-cast bf16):
    qT  [2*128, T]   Q^T per head (row block j = head j)
    kT  [128, T]     K^T of the shared kv head
    va  [128, nT*129] V chunks [128, 129] with a ones column (chunk t at cols
                     129t..129t+129); the ones column makes the PV matmul also
                     produce the softmax denominator.
  loop over k-chunks t (128 keys each), keys on PSUM partitions:
    S^T[rk, q] = kT_chunk.T @ qT  over the 640-wide valid q-span [128t, 128t+640)
    one ACT exp (scale=D^-0.5 folded in), fp32->bf16, into an SBUF ring
    triangular edge masks (diag block on DVE, border block on GpSimd)
    PV: for j=0..4, lhsT = E block (q-chunk t+j), rhs = va chunk t -> accumulate
        O_aug[q-chunk] = [128, 129] in PSUM (col 128 = denominator)
    retire q-chunk t: copy unnormalized O and denominator to SBUF staging.
  Normalization (out/den) and lse (log den) happen on the host.

The emission is software-pipelined (S matmuls run 2 k-steps ahead of exp) so
the in-order PE queue never stalls on ACT: this keeps PE busy continuously,
which also keeps the PE HAM clock-gate at 2.4 GHz.

PSUM (8 banks): 4 banks = S ring-of-3 [128, 1920]; 4 banks = O accumulators,
5 live slots with staggered lifetimes packed 2 per bank at offsets 0/129 using
(u, u+4) co-tenancy: bank u%4, offset (u//4)%2. start=True (whole-bank
has_written clear) is only ever issued by the offset-0 occupant at its first
touch, which is exactly when the other offset's previous occupant has retired.
"""

import numpy as np
import ml_dtypes

import concourse.bacc as bacc
import concourse.bass as bass
import concourse.mybir as mybir
import concourse.tile as tile
from concourse.bass_utils import run_bass_kernel_spmd

T, H, HK, D, W = 8192, 16, 4, 128, 512
NCORES = 8
SCALE = float(D) ** -0.5
BF16 = ml_dtypes.bfloat16
F32 = mybir.dt.float32
BF = mybir.dt.bfloat16

_NC_CACHE = {}


def _split_at_banks(col0, width):
    """Split [col0, col0+width) PSUM cols at 512 boundaries."""
    pieces = []
    c = col0
    end = col0 + width
    while c < end:
        nxt = min(end, (c // 512 + 1) * 512)
        pieces.append((c, nxt - c))
        c = nxt
    return pieces


def build_program(t_tokens=T):
    nT = t_tokens // 128  # number of 128-wide k/q chunks
    assert t_tokens % 512 == 0 and nT >= 8

    nc = bacc.Bacc("TRN2", target_bir_lowering=False, debug=False,
                   num_devices=NCORES)
    qT_d = nc.dram_tensor("qT", [2 * D, t_tokens], BF, kind="ExternalInput")
    kT_d = nc.dram_tensor("kT", [D, t_tokens], BF, kind="ExternalInput")
    va_d = nc.dram_tensor("va", [D, nT * 129], BF, kind="ExternalInput")
    oaug_d = nc.dram_tensor("oaug", [2, nT // 4, D, 516], F32,
                            kind="ExternalOutput")

    with tile.TileContext(nc) as tc:
        with (
            tc.tile_pool(name="resident", bufs=1) as rpool,
            tc.tile_pool(name="ostg", bufs=2) as ostg_pool,
            tc.tile_pool(name="psum", bufs=1, space="PSUM") as psum_pool,
        ):
            kT = rpool.tile([128, t_tokens], BF, tag="kT")
            va = rpool.tile([128, nT * 129], BF, tag="va")
            qT = [rpool.tile([128, t_tokens], BF, tag=f"qT{j}", name=f"qT{j}")
                  for j in range(2)]
            masks = rpool.tile([128, 256], BF, tag="masks")
            ering = rpool.tile([128, 3840], BF, tag="ering")

            sring = psum_pool.tile([128, 1920], F32, tag="sring")
            oacc = psum_pool.tile([128, 2048], F32, tag="oacc")

            # --- constant masks (bf16 0/1) ---
            # cols [0,128): diagonal block, valid iff rk <= cq
            # cols [128,256): border block, valid iff rk > cq
            nc.gpsimd.memset(masks[:, :], 1.0)
            nc.gpsimd.affine_select(
                out=masks[:, 0:128], in_=masks[:, 0:128],
                compare_op=mybir.AluOpType.is_ge, fill=0.0,
                base=0, channel_multiplier=-1, pattern=[[1, 128]],
            )
            nc.gpsimd.affine_select(
                out=masks[:, 128:256], in_=masks[:, 128:256],
                compare_op=mybir.AluOpType.is_ge, fill=0.0,
                base=-1, channel_multiplier=1, pattern=[[-1, 128]],
            )

            # --- input DMA, sliced and interleaved in first-use order ---
            ks = t_tokens // 4
            vs = (nT * 129) // 4
            qs = t_tokens // 8
            for s in range(4):
                nc.sync.dma_start(kT[:, s * ks:(s + 1) * ks],
                                  kT_d[:, s * ks:(s + 1) * ks])
                nc.sync.dma_start(qT[0][:, s * qs:(s + 1) * qs],
                                  qT_d[0:128, s * qs:(s + 1) * qs])
                nc.sync.dma_start(va[:, s * vs:(s + 1) * vs],
                                  va_d[:, s * vs:(s + 1) * vs])
            for s in range(4, 8):
                nc.sync.dma_start(qT[0][:, s * qs:(s + 1) * qs],
                                  qT_d[0:128, s * qs:(s + 1) * qs])
            for s in range(8):
                nc.sync.dma_start(qT[1][:, s * qs:(s + 1) * qs],
                                  qT_d[128:256, s * qs:(s + 1) * qs])

            def s_matmuls(j, t):
                """S^T matmuls for k-chunk t into ring slice t%3."""
                base = 640 * (t % 3)
                k0 = 128 * t
                wm = min(512, t_tokens - k0)
                for (c0, w) in _split_at_banks(base, wm):
                    qc = k0 + (c0 - base)
                    nc.tensor.matmul(sring[:, c0:c0 + w],
                                     kT[:, k0:k0 + 128],
                                     qT[j][:, qc:qc + w],
                                     start=True, stop=True)
                wt = min(128, max(0, t_tokens - k0 - 512))
                if wt > 0:
                    c0 = base + 512
                    nc.tensor.matmul(sring[:, c0:c0 + wt],
                                     kT[:, k0:k0 + 128],
                                     qT[j][:, k0 + 512:k0 + 512 + wt],
                                     start=True, stop=True)
                return wm + wt

            def width_of(t):
                k0 = 128 * t
                return min(512, t_tokens - k0) + min(128, max(0, t_tokens - k0 - 512))

            def emit_exp(t):
                sbase = 640 * (t % 3)
                ebase = 640 * (t % 6)
                w = width_of(t)
                nc.scalar.activation(ering[:, ebase:ebase + w],
                                     sring[:, sbase:sbase + w],
                                     mybir.ActivationFunctionType.Exp,
                                     scale=SCALE)

            def emit_exp_pair(t):
                # ering destinations are always adjacent for even t (ring-6);
                # sring wraps when t%3 == 2 -> two calls.
                ebase = 640 * (t % 6)
                if t % 3 <= 1:
                    sbase = 640 * (t % 3)
                    nc.scalar.activation(ering[:, ebase:ebase + 1280],
                                         sring[:, sbase:sbase + 1280],
                                         mybir.ActivationFunctionType.Exp,
                                         scale=SCALE)
                else:
                    emit_exp(t)
                    emit_exp(t + 1)

            def mask_and_pv(j, t):
                base = 640 * (t % 6)
                has_border = (128 * t + 640) <= t_tokens
                # diag mask on DVE, border mask on GpSimd; PV blocks 1-3 have
                # no mask dependency so they issue while the masks run.
                nc.vector.tensor_mul(ering[:, base:base + 128],
                                     ering[:, base:base + 128],
                                     masks[:, 0:128])
                if has_border:
                    nc.gpsimd.tensor_mul(ering[:, base + 512:base + 640],
                                         ering[:, base + 512:base + 640],
                                         masks[:, 128:256])
                jmax = min(4, nT - 1 - t)
                order = [jj for jj in (1, 2, 3) if jj <= jmax] + [0] \
                    + ([4] if jmax == 4 else [])
                # j0 stays before j4: same-bank co-tenancy requires the
                # retiring slot's last accumulation before the whole-bank
                # clearing start=True of slot u+4.
                for jj in order:
                    u = t + jj
                    off = 512 * (u % 4) + 129 * ((u // 4) % 2)
                    first = (jj == 4) or (t == 0)
                    start = first and ((u // 4) % 2 == 0)
                    # Co-tenant accumulators share banks; the sim's
                    # bank-granular group checker can't express this, but its
                    # per-byte pending-zero value model (== HW has_written)
                    # verifies the numerics.
                    nc.tensor.matmul(
                        oacc[:, off:off + 129],
                        ering[:, base + 128 * jj:base + 128 * jj + 128],
                        va[:, 129 * t:129 * t + 129],
                        start=start, stop=(jj == 0),
                        skip_group_check=True)

            ostage = [None]

            def retire(j, t):
                u = t
                off = 512 * (u % 4) + 129 * ((u // 4) % 2)
                if t % 4 == 0:
                    ostage[0] = ostg_pool.tile([128, 516], F32, tag="ostage",
                                               name="ostage")
                nc.vector.tensor_copy(
                    ostage[0][:, 129 * (t % 4):129 * (t % 4) + 129],
                    oacc[:, off:off + 129])
                if t % 4 == 3:
                    nc.sync.dma_start(oaug_d[j, t // 4], ostage[0][:, :])

            # software-pipelined emission: S runs 2 k-steps ahead of exp
            for j in range(2):
                s_matmuls(j, 0)
                s_matmuls(j, 1)
                t = 0
                while t < nT:
                    pairable = (t + 1 < nT) and (128 * (t + 1) + 640
                                                 <= t_tokens)
                    if pairable:
                        emit_exp_pair(t)
                        if t + 2 < nT:
                            s_matmuls(j, t + 2)
                        if t + 3 < nT:
                            s_matmuls(j, t + 3)
                        mask_and_pv(j, t)
                        retire(j, t)
                        mask_and_pv(j, t + 1)
                        retire(j, t + 1)
                        t += 2
                    else:
                        emit_exp(t)
                        if t + 2 < nT:
                            s_matmuls(j, t + 2)
                        mask_and_pv(j, t)
                        retire(j, t)
                        t += 1

    nc.compile()
    return nc


def _get_nc(t_tokens=T):
    if t_tokens not in _NC_CACHE:
        _NC_CACHE[t_tokens] = build_program(t_tokens)
    return _NC_CACHE[t_tokens]


def make_in_maps(query, key, value, t_tokens=T):
    q = np.asarray(query).astype(BF16).reshape(t_tokens, H, D)
    k = np.asarray(key).astype(BF16).reshape(t_tokens, HK, D)
    v = np.asarray(value).astype(BF16).reshape(t_tokens, HK, D)
    nT = t_tokens // 128
    in_maps = []
    for c in range(NCORES):
        h0, hk = 2 * c, c // 2
        qT = np.ascontiguousarray(
            q[:, h0:h0 + 2, :].transpose(1, 2, 0)).reshape(2 * D, t_tokens)
        kT = np.ascontiguousarray(k[:, hk, :].T)
        vv = v[:, hk, :].reshape(nT, 128, D).transpose(1, 0, 2)
        va = np.empty((128, nT, D + 1), dtype=BF16)
        va[:, :, :D] = vv
        va[:, :, D] = 1.0
        in_maps.append({"qT": qT, "kT": kT, "va": va.reshape(128, nT * 129)})
    return in_maps


def assemble(results, t_tokens=T):
    nT = t_tokens // 128
    out = np.empty((t_tokens, H * D), np.float32)
    lse = np.empty((H, t_tokens), np.float32)
    for c in range(NCORES):
        oaug = results[c]["oaug"]  # [2, nT//4, 128, 516]
        for j in range(2):
            a = oaug[j].reshape(nT // 4, 128, 4, 129)
            a = a.transpose(0, 2, 1, 3).reshape(t_tokens, 129)
            d_q = a[:, 128]
            cols = slice(256 * c + 128 * j, 256 * c + 128 * j + 128)
            out[:, cols] = a[:, :128] / d_q[:, None]
            lse[2 * c + j] = np.log(d_q)
    return out, lse


def kernel(query, key, value):
    nc = _get_nc(T)
    in_maps = make_in_maps(query, key, value, T)
    res = run_bass_kernel_spmd(nc, in_maps, list(range(NCORES)))
    return assemble(res.results, T)


# revision 10
# speedup vs baseline: 1.2985x; 1.0531x over previous
"""Causal sliding-window attention (T=8192, H=16, HK=4, D=128, W=512) on 8 trn2 cores.

Sharding: tensor-parallel on heads. Core c computes query heads {2c, 2c+1},
which share kv head c//2 (G = H//HK = 4, so 2 heads per core never straddle
a kv group). Each core is fully independent -- no collectives.

Per-core program (Bass/Tile, SPMD):
  inputs (host pre-transposed, pre-cast bf16):
    qT  [2*128, T]   Q^T per head (row block j = head j)
    kT  [128, T]     K^T of the shared kv head
    va  [128, nT*129] V chunks [128, 129] with a ones column (chunk t at cols
                     129t..129t+129); the ones column makes the PV matmul also
                     produce the softmax denominator.
  loop over k-chunks t (128 keys each), keys on PSUM partitions:
    S^T[rk, q] = kT_chunk.T @ qT  over the 640-wide valid q-span [128t, 128t+640)
    one ACT exp (scale=D^-0.5 folded in), fp32->bf16, into an SBUF ring
    triangular edge masks (diag block on DVE, border block on GpSimd)
    PV: for j=0..4, lhsT = E block (q-chunk t+j), rhs = va chunk t -> accumulate
        O_aug[q-chunk] = [128, 129] in PSUM (col 128 = denominator)
    retire q-chunk t: copy unnormalized O and denominator to SBUF staging.
  Normalization (out/den) and lse (log den) happen on the host.

The emission is software-pipelined (S matmuls run 2 k-steps ahead of exp) so
the in-order PE queue never stalls on ACT: this keeps PE busy continuously,
which also keeps the PE HAM clock-gate at 2.4 GHz.

PSUM (8 banks): 4 banks = S ring-of-3 [128, 1920]; 4 banks = O accumulators,
5 live slots with staggered lifetimes packed 2 per bank at offsets 0/129 using
(u, u+4) co-tenancy: bank u%4, offset (u//4)%2. start=True (whole-bank
has_written clear) is only ever issued by the offset-0 occupant at its first
touch, which is exactly when the other offset's previous occupant has retired.
"""

import numpy as np
import ml_dtypes

import concourse.bacc as bacc
import concourse.bass as bass
import concourse.mybir as mybir
import concourse.tile as tile
from concourse.bass_utils import run_bass_kernel_spmd

T, H, HK, D, W = 8192, 16, 4, 128, 512
NCORES = 8
SCALE = float(D) ** -0.5
BF16 = ml_dtypes.bfloat16
F32 = mybir.dt.float32
BF = mybir.dt.bfloat16

_NC_CACHE = {}


def _split_at_banks(col0, width):
    """Split [col0, col0+width) PSUM cols at 512 boundaries."""
    pieces = []
    c = col0
    end = col0 + width
    while c < end:
        nxt = min(end, (c // 512 + 1) * 512)
        pieces.append((c, nxt - c))
        c = nxt
    return pieces


def build_program(t_tokens=T):
    nT = t_tokens // 128  # number of 128-wide k/q chunks
    assert t_tokens % 512 == 0 and nT >= 8

    nc = bacc.Bacc("TRN2", target_bir_lowering=False, debug=False,
                   num_devices=NCORES)
    qT_d = nc.dram_tensor("qT", [2 * D, t_tokens], BF, kind="ExternalInput")
    kT_d = nc.dram_tensor("kT", [D, t_tokens], BF, kind="ExternalInput")
    va_d = nc.dram_tensor("va", [D, nT * 129], BF, kind="ExternalInput")
    oaug_d = nc.dram_tensor("oaug", [2, nT // 4, D, 516], F32,
                            kind="ExternalOutput")

    with tile.TileContext(nc) as tc:
        with (
            tc.tile_pool(name="resident", bufs=1) as rpool,
            tc.tile_pool(name="ostg", bufs=2) as ostg_pool,
            tc.tile_pool(name="psum", bufs=1, space="PSUM") as psum_pool,
        ):
            kT = rpool.tile([128, t_tokens], BF, tag="kT")
            va = rpool.tile([128, nT * 129], BF, tag="va")
            qT = [rpool.tile([128, t_tokens], BF, tag=f"qT{j}", name=f"qT{j}")
                  for j in range(2)]
            maskadd = rpool.tile([128, 256], BF, tag="maskadd")
            ident = rpool.tile([128, 128], BF, tag="ident")
            ering = rpool.tile([128, 3840], BF, tag="ering")

            sring = psum_pool.tile([128, 1920], F32, tag="sring")
            oacc = psum_pool.tile([128, 2048], F32, tag="oacc")

            # --- constants for PE-side masking ---
            # The triangular edge masks are applied as matmul accumulations
            # onto the S tiles: S_region += maskadd.T @ I adds -1e9 at the
            # invalid positions, so exp underflows them to exactly 0 and no
            # vector-engine masking is needed.
            # maskadd cols [0,128):  A[k, m] = -1e9 where m > k  (diag block)
            # maskadd cols [128,256): B[k, m] = -1e9 where m <= k (border)
            NEG = -1.0e9
            nc.gpsimd.memset(maskadd[:, :], 0.0)
            nc.gpsimd.affine_select(
                out=maskadd[:, 0:128], in_=maskadd[:, 0:128],
                compare_op=mybir.AluOpType.is_ge, fill=NEG,
                base=0, channel_multiplier=1, pattern=[[-1, 128]],
            )
            nc.gpsimd.affine_select(
                out=maskadd[:, 128:256], in_=maskadd[:, 128:256],
                compare_op=mybir.AluOpType.is_ge, fill=NEG,
                base=-1, channel_multiplier=-1, pattern=[[1, 128]],
            )
            nc.gpsimd.memset(ident[:, :], 1.0)
            nc.gpsimd.affine_select(
                out=ident[:, :], in_=ident[:, :],
                compare_op=mybir.AluOpType.is_equal, fill=0.0,
                base=0, channel_multiplier=1, pattern=[[-1, 128]],
            )

            # --- input DMA, sliced and interleaved in first-use order ---
            ks = t_tokens // 4
            vs = (nT * 129) // 4
            qs = t_tokens // 8
            for s in range(4):
                nc.sync.dma_start(kT[:, s * ks:(s + 1) * ks],
                                  kT_d[:, s * ks:(s + 1) * ks])
                nc.sync.dma_start(qT[0][:, s * qs:(s + 1) * qs],
                                  qT_d[0:128, s * qs:(s + 1) * qs])
                nc.sync.dma_start(va[:, s * vs:(s + 1) * vs],
                                  va_d[:, s * vs:(s + 1) * vs])
            for s in range(4, 8):
                nc.sync.dma_start(qT[0][:, s * qs:(s + 1) * qs],
                                  qT_d[0:128, s * qs:(s + 1) * qs])
            for s in range(8):
                nc.sync.dma_start(qT[1][:, s * qs:(s + 1) * qs],
                                  qT_d[128:256, s * qs:(s + 1) * qs])

            def s_matmuls(j, t):
                """S^T matmuls for k-chunk t into ring slice t%3.

                The first main piece and the tail piece each stay open
                (stop=False) so the triangular -1e9 mask-add matmul can
                accumulate into them and close the group.
                """
                base = 640 * (t % 3)
                k0 = 128 * t
                wm = min(512, t_tokens - k0)
                for (c0, w) in _split_at_banks(base, wm):
                    qc = k0 + (c0 - base)
                    is_first = (c0 == base)
                    nc.tensor.matmul(sring[:, c0:c0 + w],
                                     kT[:, k0:k0 + 128],
                                     qT[j][:, qc:qc + w],
                                     start=True, stop=not is_first)
                wt = min(128, max(0, t_tokens - k0 - 512))
                if wt > 0:
                    c0 = base + 512
                    nc.tensor.matmul(sring[:, c0:c0 + wt],
                                     kT[:, k0:k0 + 128],
                                     qT[j][:, k0 + 512:k0 + 512 + wt],
                                     start=True, stop=False)
                # fold the triangular edge masks in as accumulating matmuls
                nc.tensor.matmul(sring[:, base:base + 128],
                                 maskadd[:, 0:128], ident[:, :],
                                 start=False, stop=True)
                if wt > 0:
                    nc.tensor.matmul(sring[:, base + 512:base + 512 + wt],
                                     maskadd[:, 128:256], ident[:, 0:wt],
                                     start=False, stop=True)
                return wm + wt

            def width_of(t):
                k0 = 128 * t
                return min(512, t_tokens - k0) + min(128, max(0, t_tokens - k0 - 512))

            def emit_exp(t):
                sbase = 640 * (t % 3)
                ebase = 640 * (t % 6)
                w = width_of(t)
                nc.scalar.activation(ering[:, ebase:ebase + w],
                                     sring[:, sbase:sbase + w],
                                     mybir.ActivationFunctionType.Exp,
                                     scale=SCALE)

            def emit_exp_pair(t):
                # ering destinations are always adjacent for even t (ring-6);
                # sring wraps when t%3 == 2 -> two calls.
                ebase = 640 * (t % 6)
                if t % 3 <= 1:
                    sbase = 640 * (t % 3)
                    nc.scalar.activation(ering[:, ebase:ebase + 1280],
                                         sring[:, sbase:sbase + 1280],
                                         mybir.ActivationFunctionType.Exp,
                                         scale=SCALE)
                else:
                    emit_exp(t)
                    emit_exp(t + 1)

            def mask_and_pv(j, t):
                base = 640 * (t % 6)
                jmax = min(4, nT - 1 - t)
                # ascending j: same-bank co-tenancy requires the retiring
                # slot's last accumulation (j0) before the whole-bank
                # clearing start=True of slot u+4 (j4).
                for jj in range(jmax + 1):
                    u = t + jj
                    off = 512 * (u % 4) + 129 * ((u // 4) % 2)
                    first = (jj == 4) or (t == 0)
                    start = first and ((u // 4) % 2 == 0)
                    # Co-tenant accumulators share banks; the sim's
                    # bank-granular group checker can't express this, but its
                    # per-byte pending-zero value model (== HW has_written)
                    # verifies the numerics.
                    nc.tensor.matmul(
                        oacc[:, off:off + 129],
                        ering[:, base + 128 * jj:base + 128 * jj + 128],
                        va[:, 129 * t:129 * t + 129],
                        start=start, stop=(jj == 0),
                        skip_group_check=True)

            ostage = [None]

            def retire(j, t):
                u = t
                off = 512 * (u % 4) + 129 * ((u // 4) % 2)
                if t % 4 == 0:
                    ostage[0] = ostg_pool.tile([128, 516], F32, tag="ostage",
                                               name="ostage")
                nc.vector.tensor_copy(
                    ostage[0][:, 129 * (t % 4):129 * (t % 4) + 129],
                    oacc[:, off:off + 129])
                if t % 4 == 3:
                    nc.sync.dma_start(oaug_d[j, t // 4], ostage[0][:, :])

            # software-pipelined emission: S runs 2 k-steps ahead of exp
            for j in range(2):
                s_matmuls(j, 0)
                s_matmuls(j, 1)
                t = 0
                while t < nT:
                    pairable = (t + 1 < nT) and (128 * (t + 1) + 640
                                                 <= t_tokens)
                    if pairable:
                        emit_exp_pair(t)
                        if t + 2 < nT:
                            s_matmuls(j, t + 2)
                        if t + 3 < nT:
                            s_matmuls(j, t + 3)
                        mask_and_pv(j, t)
                        retire(j, t)
                        mask_and_pv(j, t + 1)
                        retire(j, t + 1)
                        t += 2
                    else:
                        emit_exp(t)
                        if t + 2 < nT:
                            s_matmuls(j, t + 2)
                        mask_and_pv(j, t)
                        retire(j, t)
                        t += 1

    nc.compile()
    return nc


def _get_nc(t_tokens=T):
    if t_tokens not in _NC_CACHE:
        _NC_CACHE[t_tokens] = build_program(t_tokens)
    return _NC_CACHE[t_tokens]


def make_in_maps(query, key, value, t_tokens=T):
    q = np.asarray(query).astype(BF16).reshape(t_tokens, H, D)
    k = np.asarray(key).astype(BF16).reshape(t_tokens, HK, D)
    v = np.asarray(value).astype(BF16).reshape(t_tokens, HK, D)
    nT = t_tokens // 128
    in_maps = []
    for c in range(NCORES):
        h0, hk = 2 * c, c // 2
        qT = np.ascontiguousarray(
            q[:, h0:h0 + 2, :].transpose(1, 2, 0)).reshape(2 * D, t_tokens)
        kT = np.ascontiguousarray(k[:, hk, :].T)
        vv = v[:, hk, :].reshape(nT, 128, D).transpose(1, 0, 2)
        va = np.empty((128, nT, D + 1), dtype=BF16)
        va[:, :, :D] = vv
        va[:, :, D] = 1.0
        in_maps.append({"qT": qT, "kT": kT, "va": va.reshape(128, nT * 129)})
    return in_maps


def assemble(results, t_tokens=T):
    nT = t_tokens // 128
    out = np.empty((t_tokens, H * D), np.float32)
    lse = np.empty((H, t_tokens), np.float32)
    for c in range(NCORES):
        oaug = results[c]["oaug"]  # [2, nT//4, 128, 516]
        for j in range(2):
            a = oaug[j].reshape(nT // 4, 128, 4, 129)
            a = a.transpose(0, 2, 1, 3).reshape(t_tokens, 129)
            d_q = a[:, 128]
            cols = slice(256 * c + 128 * j, 256 * c + 128 * j + 128)
            out[:, cols] = a[:, :128] / d_q[:, None]
            lse[2 * c + j] = np.log(d_q)
    return out, lse


def kernel(query, key, value):
    nc = _get_nc(T)
    in_maps = make_in_maps(query, key, value, T)
    res = run_bass_kernel_spmd(nc, in_maps, list(range(NCORES)))
    return assemble(res.results, T)
